# revision 50
# baseline (speedup 1.0000x reference)
"""Exponentiated-quadratic (RBF) kernel matrix on 8 Trainium2 NeuronCores.

K[i, j] = sigma * exp(-0.5 * ||x1_i/rho - x2_j/rho||^2)
        with sigma = exp(log_sigma)^2, rho = exp(log_rho)

Strategy
--------
Row-shard x1 across the 8 cores (512 rows each), replicate x2. Each core
computes S = (x1/rho) @ (x2/rho)^T - 0.5*||y_j||^2 on the tensor engine and
finishes with one ScalarE activation per PSUM tile:
K = exp(S + (-0.5*||x_i||^2 + 2*log_sigma)), using ACT's free per-partition
bias (exact fp32 for the x-norms) — so the whole epilogue is a single pass.

Matmul precision: 3-pass bf16 split (Ah.Bh + Ah.Bl + Al.Bh, fp32 PSUM
accumulation, the dropped Al.Bl term is ~2^-18 relative) plus a K=3
ones-weighted pass adding the triple-bf16-split -0.5*||y_j||^2 row. Measured
~2.6e-5 scale-relative output error.

PE utilisation: K=32 fits a 32-row strip of the 128x128 array, so four
matmul streams run CONCURRENTLY via tile_position=(32s, 0) — column slice q
of each PSUM tile runs in strip q. This quadruples matmul throughput and
lets each strip's LDWEIGHTS prefetch under the other strips' matmuls.

The 4-strip layout also makes every input DMA a dense 128-partition
transfer (the fast path: ~300 GB/s vs ~50 GB/s for a 34-partition load): B
is packed on the host so strip s's rows hold exactly the columns strip s
consumes (no duplication), A is replicated per strip, and the fp32 ACT bias
rides along bit-cast as bf16 column pairs. The two input loads go on the
two parallel HWDGE rings (sync + scalar); output stores alternate between
the rings, and the last two row-blocks store in PSW halves right after
each activation so the tail transfers start as early as possible.

walrus in this container rejects instructions carrying more than one
semaphore wait, which shapes several things: single fused input tensors
(first matmul = one wait), persistent PSUM tiles (pool re-allocation adds a
same-engine PE wait), total HWDGE DMA count <= 8 (lane reuse adds an
ordering wait), ACT->ACT pseudo-deps demoted to nosync, and a chain of
single-wait NOPs on the sync sequencer that "observes" every completion
before the framework's kernel-tail drain (the nops nosync-order after all
DMAs so none head-of-line blocks a pending store).
"""

import numpy as np
import ml_dtypes

import concourse.bass as bass
import concourse.mybir as mybir
import concourse.tile as tile
from concourse.bass_utils import run_bass_kernel_spmd
from concourse.tile import add_dep_helper

N, M, P = 4096, 4096, 32
NCORES = 8
NSHARD = N // NCORES  # 512 rows of x1 per core
IBLK = 128            # output row-block = PSUM partition dim
JBLK = 512            # matmul free dim = one fp32 PSUM bank
PSW = 2048            # PSUM tile width (4 banks) = one exp-activation
NSTRIP = 4            # concurrent PE row strips (K=32 each)
NI = NSHARD // IBLK   # 4 row-blocks
NH = M // PSW         # 2 PSUM tiles per row-block

BF16 = mybir.dt.bfloat16
NPBF16 = ml_dtypes.bfloat16

# load1 column layout (all bf16, 128 partitions). Only row-blocks i=0,1 of
# A ride in l1 (the critical first load); i=2,3 slices arrive with l2,
# which lands well before they are consumed (~20us in).
#   [A_hi(i01) 256 | A_lo(i01) 256 | Bh(h=0) 512 | Bl(h=0) 512 |
#    Yn(h=0) 512 | ones 128 | xn_bits 8 | pad 56]             -> 2240 cols
# load2: [Bh(h=1) 512 | Bl(h=1) 512 | Yn(h=1) 512 |
#    A_hi(i23) 256 | A_lo(i23) 256 | pad 64]                  -> 2112 cols
AHI_O = 0
ALO_O = 256
BH0_O = 512
BL0_O = 1024
YN0_O = 1536
ONES_O = 2048
XN_O = 2176
L1_W = 2240
AHI2_O = 1536  # in l2, after the three 512-wide B/Yn regions
ALO2_O = 1792
L2_W = 2112


def _build_nc():
    nc = bass.Bass()
    l1_t = nc.declare_dram_parameter("l1_t", [IBLK, L1_W], BF16, isOutput=False)
    l2_t = nc.declare_dram_parameter("l2_t", [IBLK, L2_W], BF16, isOutput=False)
    out = nc.declare_dram_parameter("out", [NSHARD, M], mybir.dt.float32, isOutput=True)

    with tile.TileContext(nc) as tc:
        with (
            tc.tile_pool(name="inp", bufs=1) as inp_pool,
            tc.tile_pool(name="stage", bufs=1) as stage_pool,
            tc.tile_pool(name="ps", bufs=1, space="PSUM") as ps_pool,
        ):
            dma_insts = []
            l1_sb = inp_pool.tile([IBLK, L1_W], BF16, tag="l1")
            dma_insts.append(nc.sync.dma_start(out=l1_sb, in_=l1_t[:, :]))
            l2_sb = inp_pool.tile([IBLK, L2_W], BF16, tag="l2")
            dma_insts.append(nc.scalar.dma_start(out=l2_sb, in_=l2_t[:, :]))

            def rows(s, k=32):
                return slice(32 * s, 32 * s + k)

            def bh(h, s):
                sb, o = (l1_sb, BH0_O) if h == 0 else (l2_sb, 0)
                return sb[rows(s), o : o + JBLK]

            def bl(h, s):
                sb, o = (l1_sb, BL0_O) if h == 0 else (l2_sb, JBLK)
                return sb[rows(s), o : o + JBLK]

            def ynr(h, s):
                sb, o = (l1_sb, YN0_O) if h == 0 else (l2_sb, 2 * JBLK)
                return sb[rows(s, 3), o : o + JBLK]

            xn_bias = l1_sb[:, XN_O : XN_O + 2 * NI].bitcast(mybir.dt.float32)

            # Tiny ACT-engine read of l1 so the scalar engine observes the l1
            # DMA semaphore here (1 wait); the real activations then carry
            # only their PE wait (walrus rejects multi-wait ACTIVATE, and
            # Tile doesn't track that the PE wait transitively covers l1).
            scratch = inp_pool.tile([IBLK, 1], mybir.dt.float32, tag="scr")
            nc.scalar.copy(out=scratch, in_=l1_sb[:, 0:1])

            ps_tiles = [
                ps_pool.tile(
                    [IBLK, PSW], mybir.dt.float32, tag=f"ps{h}", name=f"ps{h}"
                )
                for h in range(NH)
            ]

            act_insts = []
            mm_insts = []
            for i in range(NI):
                out_sb = stage_pool.tile(
                    [IBLK, M], mybir.dt.float32, tag=f"out{i}", name=f"out{i}"
                )
                for h in range(NH):
                    ps = ps_tiles[h]
                    # 4 passes x 4 strips; strip s = column slice q=s of the
                    # PSUM tile. Inner loop cycles strips so consecutive
                    # matmuls run in different row groups (concurrent).
                    if i < 2:
                        ahi_sb, ahi_o = l1_sb, AHI_O + i * IBLK
                        alo_sb, alo_o = l1_sb, ALO_O + i * IBLK
                    else:
                        ahi_sb, ahi_o = l2_sb, AHI2_O + (i - 2) * IBLK
                        alo_sb, alo_o = l2_sb, ALO2_O + (i - 2) * IBLK
                    for p in range(4):
                        start = p == 0
                        stop = p == 3
                        for s in range(NSTRIP):
                            if p == 0:
                                lhsT = ahi_sb[rows(s), ahi_o : ahi_o + IBLK]
                                rhs = bh(h, s)
                            elif p == 1:
                                lhsT = ahi_sb[rows(s), ahi_o : ahi_o + IBLK]
                                rhs = bl(h, s)
                            elif p == 2:
                                lhsT = alo_sb[rows(s), alo_o : alo_o + IBLK]
                                rhs = bh(h, s)
                            else:
                                lhsT = l1_sb[rows(s, 3), ONES_O : ONES_O + IBLK]
                                rhs = ynr(h, s)
                            mm_insts.append(
                                nc.tensor.matmul(
                                    ps[:, s * JBLK : (s + 1) * JBLK],
                                    lhsT=lhsT,
                                    rhs=rhs,
                                    start=start,
                                    stop=stop,
                                    tile_position=(32 * s, 0),
                                )
                            )
                    act_insts.append(
                        nc.scalar.activation(
                            out=out_sb[:, h * PSW : (h + 1) * PSW],
                            in_=ps,
                            func=mybir.ActivationFunctionType.Exp,
                            bias=xn_bias[:, i : i + 1],
                            scale=1.0,
                        )
                    )
                    # Last two row-blocks: store each PSW half as soon as its
                    # activation lands (the early blocks' full-width stores
                    # already overlap compute; the tail ones wouldn't). Total
                    # DMA count stays at 8 = one per HWDGE lane.
                    if i >= NI - 2:
                        eng = nc.sync if (i + h) % 2 == 0 else nc.scalar
                        dma_insts.append(
                            eng.dma_start(
                                out=out[
                                    i * IBLK : (i + 1) * IBLK,
                                    h * PSW : (h + 1) * PSW,
                                ],
                                in_=out_sb[:, h * PSW : (h + 1) * PSW],
                            )
                        )
                if i < NI - 2:
                    eng = nc.sync if i % 2 == 0 else nc.scalar
                    dma_insts.append(
                        eng.dma_start(
                            out=out[i * IBLK : (i + 1) * IBLK, :], in_=out_sb
                        )
                    )

            # Demote ACT->ACT pseudo-deps (PSUM bank read-read serialization,
            # already ordered through the interleaved matmuls + same-engine
            # FIFO) to nosync: walrus rejects multi-wait ACTIVATE.
            import bass_rust as _br

            act_names = {a.ins.name for a in act_insts}
            for a in act_insts:
                deps = list(a.ins.sync_dependency_names())
                spurious = [d for d in deps if d in act_names]
                if spurious:
                    keep = [d for d in deps if d not in act_names]
                    a.ins.take_sync_dependencies()
                    a.ins.set_sync_dependencies(
                        _br.InstructionNameOrderedSet(keep)
                    )
                    a.ins.add_nosync_dependencies_from(
                        _br.InstructionNameOrderedSet(spurious)
                    )

            # Wait-funnel so the framework's kernel-tail drain needs no waits
            # of its own (walrus rejects its usual all-sems wait list). Each
            # nop also nosync-orders after every DMA so the scheduler cannot
            # slot a slow-waiting nop ahead of a still-pending store on the
            # same queue (head-of-line blocking).
            for t in [mm_insts[-1], act_insts[-1], *dma_insts]:
                nop = nc.sync.nop(nofuse=True, hint="tail_funnel")
                add_dep_helper(nop.ins, t.ins, True, "tail wait funnel")
                for dd in dma_insts:
                    if dd is not t:
                        add_dep_helper(nop.ins, dd.ins, False, "funnel order")
    return nc


def _bf16_splits(x, n):
    """Split fp32 array into n bf16 parts summing to ~x."""
    parts = []
    rem = x.astype(np.float32)
    for _ in range(n):
        p = rem.astype(NPBF16)
        parts.append(p)
        rem = rem - p.astype(np.float32)
    return parts


def run(x1, x2, log_rho, log_sigma, trace=False):
    """Returns (K, exec_time_ns). exec_time_ns is None unless trace=True."""
    x1 = np.asarray(x1, dtype=np.float32)
    x2 = np.asarray(x2, dtype=np.float32)
    rho = float(np.exp(np.float64(np.asarray(log_rho))))
    log_sig = 2.0 * float(np.asarray(log_sigma))  # log(sigma)

    xs = (x1 / np.float32(rho)).astype(np.float32)
    ys = (x2 / np.float32(rho)).astype(np.float32)
    xn = np.einsum("np,np->n", xs, xs, dtype=np.float64)
    yn = np.einsum("mp,mp->m", ys, ys, dtype=np.float64)

    a = xs.T.astype(np.float32)  # (32, N)
    b = ys.T.astype(np.float32)  # (32, M)
    a_hi, a_lo = _bf16_splits(a, 2)
    b_hi, b_lo = _bf16_splits(b, 2)
    y1, y2, y3 = _bf16_splits((-0.5 * yn).astype(np.float32), 3)
    # per-row ACT bias: -0.5*||x_i||^2 + log(sigma), exact fp32
    xbias = ((-0.5 * xn) + log_sig).astype(np.float32)

    def pack_b(src, h):
        # strip s rows hold the columns strip s consumes: B[:, h*PSW+s*JBLK..]
        o = np.zeros((IBLK, JBLK), NPBF16)
        for s in range(NSTRIP):
            o[32 * s : 32 * s + 32] = src[:, h * PSW + s * JBLK : h * PSW + (s + 1) * JBLK]
        return o

    def pack_yn(h):
        o = np.zeros((IBLK, JBLK), NPBF16)
        for s in range(NSTRIP):
            for r, yr in enumerate((y1, y2, y3)):
                o[32 * s + r] = yr[h * PSW + s * JBLK : h * PSW + (s + 1) * JBLK]
        return o

    ones = np.zeros((IBLK, IBLK), NPBF16)
    for s in range(NSTRIP):
        ones[32 * s : 32 * s + 3] = NPBF16(1.0)

    l2base = np.zeros((IBLK, L2_W), NPBF16)
    l2base[:, 0:JBLK] = pack_b(b_hi, 1)
    l2base[:, JBLK : 2 * JBLK] = pack_b(b_lo, 1)
    l2base[:, 2 * JBLK : 3 * JBLK] = pack_yn(1)

    half = 2 * IBLK  # 256 cols of A per tensor (two row-blocks)
    nc = _build_nc()
    in_maps = []
    for c in range(NCORES):
        sl0 = slice(c * NSHARD, c * NSHARD + half)
        sl2 = slice(c * NSHARD + half, (c + 1) * NSHARD)
        l1 = np.zeros((IBLK, L1_W), NPBF16)
        l2 = l2base.copy()
        for s in range(NSTRIP):
            r = slice(32 * s, 32 * s + 32)
            l1[r, AHI_O : AHI_O + half] = a_hi[:, sl0]
            l1[r, ALO_O : ALO_O + half] = a_lo[:, sl0]
            l2[r, AHI2_O : AHI2_O + half] = a_hi[:, sl2]
            l2[r, ALO2_O : ALO2_O + half] = a_lo[:, sl2]
        l1[:, BH0_O : BH0_O + JBLK] = pack_b(b_hi, 0)
        l1[:, BL0_O : BL0_O + JBLK] = pack_b(b_lo, 0)
        l1[:, YN0_O : YN0_O + JBLK] = pack_yn(0)
        l1[:, ONES_O : ONES_O + IBLK] = ones
        # fp32 bias bits ride along as bf16 column pairs
        xb = np.zeros((IBLK, NI), np.float32)
        for i in range(NI):
            xb[:, i] = xbias[c * NSHARD + i * IBLK : c * NSHARD + (i + 1) * IBLK]
        l1[:, XN_O : XN_O + 2 * NI] = xb.view(np.uint16).view(NPBF16)
        in_maps.append(
            {
                "l1_t": np.ascontiguousarray(l1),
                "l2_t": np.ascontiguousarray(l2),
            }
        )

    res = run_bass_kernel_spmd(
        nc, in_maps, core_ids=list(range(NCORES)), trace=trace
    )
    full = np.concatenate(
        [res.results[c]["out"] for c in range(NCORES)], axis=0
    )
    return full, res.exec_time_ns


def kernel(x1, x2, log_rho, log_sigma):
    out, _ = run(x1, x2, log_rho, log_sigma, trace=False)
    return out


# revision 51
# speedup vs baseline: 1.1115x; 1.1115x over previous
"""Exponentiated-quadratic (RBF) kernel matrix on 8 Trainium2 NeuronCores.

K[i, j] = sigma * exp(-0.5 * ||x1_i/rho - x2_j/rho||^2)
        with sigma = exp(log_sigma)^2, rho = exp(log_rho)

Strategy
--------
Row-shard x1 across the 8 cores (512 rows each), replicate x2. Each core
computes S = (x1/rho) @ (x2/rho)^T - 0.5*||y_j||^2 on the tensor engine and
finishes with one ScalarE activation per PSUM tile:
K = exp(S + (-0.5*||x_i||^2 + 2*log_sigma)), using ACT's free per-partition
bias (exact fp32 for the x-norms) — so the whole epilogue is a single pass.

Matmul precision: 3-pass bf16 split (Ah.Bh + Ah.Bl + Al.Bh, fp32 PSUM
accumulation, the dropped Al.Bl term is ~2^-18 relative) plus a K=3
ones-weighted pass adding the triple-bf16-split -0.5*||y_j||^2 row. Measured
~2.6e-5 scale-relative output error.

PE utilisation: K=32 fits a 32-row strip of the 128x128 array, so four
matmul streams run CONCURRENTLY via tile_position=(32s, 0) — column slice q
of each PSUM tile runs in strip q. This quadruples matmul throughput and
lets each strip's LDWEIGHTS prefetch under the other strips' matmuls.

The 4-strip layout also makes every input DMA a dense 128-partition
transfer (the fast path: ~300 GB/s vs ~50 GB/s for a 34-partition load): B
is packed on the host so strip s's rows hold exactly the columns strip s
consumes (no duplication), A is replicated per strip, and the fp32 ACT bias
rides along bit-cast as bf16 column pairs. The two input loads go on the
two parallel HWDGE rings (sync + scalar); output stores alternate between
the rings, and the last two row-blocks store in PSW halves right after
each activation so the tail transfers start as early as possible.

walrus in this container rejects instructions carrying more than one
semaphore wait, which shapes several things: single fused input tensors
(first matmul = one wait), persistent PSUM tiles (pool re-allocation adds a
same-engine PE wait), total HWDGE DMA count <= 8 (lane reuse adds an
ordering wait), ACT->ACT pseudo-deps demoted to nosync, and a chain of
single-wait NOPs on the sync sequencer that "observes" every completion
before the framework's kernel-tail drain (the nops nosync-order after all
DMAs so none head-of-line blocks a pending store).
"""

import numpy as np
import ml_dtypes

import concourse.bass as bass
import concourse.mybir as mybir
import concourse.tile as tile
from concourse.bass_utils import run_bass_kernel_spmd
from concourse.tile import add_dep_helper

N, M, P = 4096, 4096, 32
NCORES = 8
NSHARD = N // NCORES  # 512 rows of x1 per core
IBLK = 128            # output row-block = PSUM partition dim
JBLK = 512            # matmul free dim = one fp32 PSUM bank
PSW = 2048            # PSUM tile width (4 banks) = one exp-activation
NSTRIP = 4            # concurrent PE row strips (K=32 each)
NI = NSHARD // IBLK   # 4 row-blocks
NH = M // PSW         # 2 PSUM tiles per row-block

BF16 = mybir.dt.bfloat16
NPBF16 = ml_dtypes.bfloat16

# load1 column layout (all bf16, 128 partitions):
#   [A_hi 512 | A_lo 512 | Bh(h=0) 512 | Bl(h=0) 512 | Yn(h=0) 512 |
#    ones 128 | xn_bits 8 | pad 56]                           -> 2752 cols
# load2: [Bh(h=1) 512 | Bl(h=1) 512 | Yn(h=1) 512 | pad 64]   -> 1600 cols
AHI_O = 0
ALO_O = 512
BH0_O = 1024
BL0_O = 1536
YN0_O = 2048
ONES_O = 2560
XN_O = 2688
L1_W = 2752
L2_W = 1600


def _build_nc():
    nc = bass.Bass()
    l1_t = nc.declare_dram_parameter("l1_t", [IBLK, L1_W], BF16, isOutput=False)
    l2_t = nc.declare_dram_parameter("l2_t", [IBLK, L2_W], BF16, isOutput=False)
    out = nc.declare_dram_parameter("out", [NSHARD, M], mybir.dt.float32, isOutput=True)

    with tile.TileContext(nc) as tc:
        with (
            tc.tile_pool(name="inp", bufs=1) as inp_pool,
            tc.tile_pool(name="stage", bufs=1) as stage_pool,
            tc.tile_pool(name="ps", bufs=1, space="PSUM") as ps_pool,
        ):
            dma_insts = []
            l1_sb = inp_pool.tile([IBLK, L1_W], BF16, tag="l1")
            dma_insts.append(nc.sync.dma_start(out=l1_sb, in_=l1_t[:, :]))
            l2_sb = inp_pool.tile([IBLK, L2_W], BF16, tag="l2")
            dma_insts.append(nc.scalar.dma_start(out=l2_sb, in_=l2_t[:, :]))

            def rows(s, k=32):
                return slice(32 * s, 32 * s + k)

            def bh(h, s):
                sb, o = (l1_sb, BH0_O) if h == 0 else (l2_sb, 0)
                return sb[rows(s), o : o + JBLK]

            def bl(h, s):
                sb, o = (l1_sb, BL0_O) if h == 0 else (l2_sb, JBLK)
                return sb[rows(s), o : o + JBLK]

            def ynr(h, s):
                sb, o = (l1_sb, YN0_O) if h == 0 else (l2_sb, 2 * JBLK)
                return sb[rows(s, 3), o : o + JBLK]

            xn_bias = l1_sb[:, XN_O : XN_O + 2 * NI].bitcast(mybir.dt.float32)

            # Tiny ACT-engine read of l1 so the scalar engine observes the l1
            # DMA semaphore here (1 wait); the real activations then carry
            # only their PE wait (walrus rejects multi-wait ACTIVATE, and
            # Tile doesn't track that the PE wait transitively covers l1).
            scratch = inp_pool.tile([IBLK, 1], mybir.dt.float32, tag="scr")
            nc.scalar.copy(out=scratch, in_=l1_sb[:, 0:1])

            ps_tiles = [
                ps_pool.tile(
                    [IBLK, PSW], mybir.dt.float32, tag=f"ps{h}", name=f"ps{h}"
                )
                for h in range(NH)
            ]

            act_insts = []
            mm_insts = []
            for i in range(NI):
                out_sb = stage_pool.tile(
                    [IBLK, M], mybir.dt.float32, tag=f"out{i}", name=f"out{i}"
                )
                for h in range(NH):
                    ps = ps_tiles[h]
                    # 4 passes x 4 strips; strip s = column slice q=s of the
                    # PSUM tile. Inner loop cycles strips so consecutive
                    # matmuls run in different row groups (concurrent).
                    for p in range(4):
                        start = p == 0
                        stop = p == 3
                        for s in range(NSTRIP):
                            if p == 0:
                                lhsT = l1_sb[rows(s), AHI_O + i * IBLK : AHI_O + (i + 1) * IBLK]
                                rhs = bh(h, s)
                            elif p == 1:
                                lhsT = l1_sb[rows(s), AHI_O + i * IBLK : AHI_O + (i + 1) * IBLK]
                                rhs = bl(h, s)
                            elif p == 2:
                                lhsT = l1_sb[rows(s), ALO_O + i * IBLK : ALO_O + (i + 1) * IBLK]
                                rhs = bh(h, s)
                            else:
                                lhsT = l1_sb[rows(s, 3), ONES_O : ONES_O + IBLK]
                                rhs = ynr(h, s)
                            mm_insts.append(
                                nc.tensor.matmul(
                                    ps[:, s * JBLK : (s + 1) * JBLK],
                                    lhsT=lhsT,
                                    rhs=rhs,
                                    start=start,
                                    stop=stop,
                                    tile_position=(32 * s, 0),
                                )
                            )
                    act_insts.append(
                        nc.scalar.activation(
                            out=out_sb[:, h * PSW : (h + 1) * PSW],
                            in_=ps,
                            func=mybir.ActivationFunctionType.Exp,
                            bias=xn_bias[:, i : i + 1],
                            scale=1.0,
                        )
                    )
                    # Last two row-blocks: store each PSW half as soon as its
                    # activation lands (the early blocks' full-width stores
                    # already overlap compute; the tail ones wouldn't). Total
                    # DMA count stays at 8 = one per HWDGE lane.
                    if i >= NI - 2:
                        eng = nc.sync if (i + h) % 2 == 0 else nc.scalar
                        dma_insts.append(
                            eng.dma_start(
                                out=out[
                                    i * IBLK : (i + 1) * IBLK,
                                    h * PSW : (h + 1) * PSW,
                                ],
                                in_=out_sb[:, h * PSW : (h + 1) * PSW],
                            )
                        )
                if i < NI - 2:
                    eng = nc.sync if i % 2 == 0 else nc.scalar
                    dma_insts.append(
                        eng.dma_start(
                            out=out[i * IBLK : (i + 1) * IBLK, :], in_=out_sb
                        )
                    )

            # Demote ACT->ACT pseudo-deps (PSUM bank read-read serialization,
            # already ordered through the interleaved matmuls + same-engine
            # FIFO) to nosync: walrus rejects multi-wait ACTIVATE.
            import bass_rust as _br

            act_names = {a.ins.name for a in act_insts}
            for a in act_insts:
                deps = list(a.ins.sync_dependency_names())
                spurious = [d for d in deps if d in act_names]
                if spurious:
                    keep = [d for d in deps if d not in act_names]
                    a.ins.take_sync_dependencies()
                    a.ins.set_sync_dependencies(
                        _br.InstructionNameOrderedSet(keep)
                    )
                    a.ins.add_nosync_dependencies_from(
                        _br.InstructionNameOrderedSet(spurious)
                    )

            # Wait-funnel so the framework's kernel-tail drain needs no waits
            # of its own (walrus rejects its usual all-sems wait list). Each
            # nop also nosync-orders after every DMA so the scheduler cannot
            # slot a slow-waiting nop ahead of a still-pending store on the
            # same queue (head-of-line blocking).
            for t in [mm_insts[-1], act_insts[-1], *dma_insts]:
                nop = nc.sync.nop(nofuse=True, hint="tail_funnel")
                add_dep_helper(nop.ins, t.ins, True, "tail wait funnel")
                for dd in dma_insts:
                    if dd is not t:
                        add_dep_helper(nop.ins, dd.ins, False, "funnel order")
    return nc


def _bf16_splits(x, n):
    """Split fp32 array into n bf16 parts summing to ~x."""
    parts = []
    rem = x.astype(np.float32)
    for _ in range(n):
        p = rem.astype(NPBF16)
        parts.append(p)
        rem = rem - p.astype(np.float32)
    return parts


def run(x1, x2, log_rho, log_sigma, trace=False):
    """Returns (K, exec_time_ns). exec_time_ns is None unless trace=True."""
    x1 = np.asarray(x1, dtype=np.float32)
    x2 = np.asarray(x2, dtype=np.float32)
    rho = float(np.exp(np.float64(np.asarray(log_rho))))
    log_sig = 2.0 * float(np.asarray(log_sigma))  # log(sigma)

    xs = (x1 / np.float32(rho)).astype(np.float32)
    ys = (x2 / np.float32(rho)).astype(np.float32)
    xn = np.einsum("np,np->n", xs, xs, dtype=np.float64)
    yn = np.einsum("mp,mp->m", ys, ys, dtype=np.float64)

    a = xs.T.astype(np.float32)  # (32, N)
    b = ys.T.astype(np.float32)  # (32, M)
    a_hi, a_lo = _bf16_splits(a, 2)
    b_hi, b_lo = _bf16_splits(b, 2)
    y1, y2, y3 = _bf16_splits((-0.5 * yn).astype(np.float32), 3)
    # per-row ACT bias: -0.5*||x_i||^2 + log(sigma), exact fp32
    xbias = ((-0.5 * xn) + log_sig).astype(np.float32)

    def pack_b(src, h):
        # strip s rows hold the columns strip s consumes: B[:, h*PSW+s*JBLK..]
        o = np.zeros((IBLK, JBLK), NPBF16)
        for s in range(NSTRIP):
            o[32 * s : 32 * s + 32] = src[:, h * PSW + s * JBLK : h * PSW + (s + 1) * JBLK]
        return o

    def pack_yn(h):
        o = np.zeros((IBLK, JBLK), NPBF16)
        for s in range(NSTRIP):
            for r, yr in enumerate((y1, y2, y3)):
                o[32 * s + r] = yr[h * PSW + s * JBLK : h * PSW + (s + 1) * JBLK]
        return o

    ones = np.zeros((IBLK, IBLK), NPBF16)
    for s in range(NSTRIP):
        ones[32 * s : 32 * s + 3] = NPBF16(1.0)

    l2 = np.zeros((IBLK, L2_W), NPBF16)
    l2[:, 0:JBLK] = pack_b(b_hi, 1)
    l2[:, JBLK : 2 * JBLK] = pack_b(b_lo, 1)
    l2[:, 2 * JBLK : 3 * JBLK] = pack_yn(1)

    nc = _build_nc()
    in_maps = []
    for c in range(NCORES):
        sl = slice(c * NSHARD, (c + 1) * NSHARD)
        l1 = np.zeros((IBLK, L1_W), NPBF16)
        for s in range(NSTRIP):
            l1[32 * s : 32 * s + 32, AHI_O : AHI_O + NSHARD] = a_hi[:, sl]
            l1[32 * s : 32 * s + 32, ALO_O : ALO_O + NSHARD] = a_lo[:, sl]
        l1[:, BH0_O : BH0_O + JBLK] = pack_b(b_hi, 0)
        l1[:, BL0_O : BL0_O + JBLK] = pack_b(b_lo, 0)
        l1[:, YN0_O : YN0_O + JBLK] = pack_yn(0)
        l1[:, ONES_O : ONES_O + IBLK] = ones
        # fp32 bias bits ride along as bf16 column pairs
        xb = np.zeros((IBLK, NI), np.float32)
        for i in range(NI):
            xb[:, i] = xbias[c * NSHARD + i * IBLK : c * NSHARD + (i + 1) * IBLK]
        l1[:, XN_O : XN_O + 2 * NI] = xb.view(np.uint16).view(NPBF16)
        in_maps.append({"l1_t": np.ascontiguousarray(l1), "l2_t": l2})

    res = run_bass_kernel_spmd(
        nc, in_maps, core_ids=list(range(NCORES)), trace=trace
    )
    full = np.concatenate(
        [res.results[c]["out"] for c in range(NCORES)], axis=0
    )
    return full, res.exec_time_ns


def kernel(x1, x2, log_rho, log_sigma):
    out, _ = run(x1, x2, log_rho, log_sigma, trace=False)
    return out


# revision 56
# speedup vs baseline: 1.1725x; 1.0549x over previous
"""Exponentiated-quadratic (RBF) kernel matrix on 8 Trainium2 NeuronCores.

K[i, j] = sigma * exp(-0.5 * ||x1_i/rho - x2_j/rho||^2)
        with sigma = exp(log_sigma)^2, rho = exp(log_rho)

Strategy
--------
Row-shard x1 across the 8 cores (512 rows each), replicate x2. Each core
computes S = (x1/rho) @ (x2/rho)^T - 0.5*||y_j||^2 on the tensor engine and
finishes with one ScalarE activation per PSUM tile:
K = exp(S + (-0.5*||x_i||^2 + 2*log_sigma)), using ACT's free per-partition
bias (exact fp32 for the x-norms) — so the whole epilogue is a single pass.

Matmul precision: 3-pass bf16 split (Ah.Bh + Ah.Bl + Al.Bh, fp32 PSUM
accumulation, the dropped Al.Bl term is ~2^-18 relative) plus a K=3
ones-weighted pass adding the triple-bf16-split -0.5*||y_j||^2 row. Measured
~2.6e-5 scale-relative output error.

PE utilisation: K=32 fits a 32-row strip of the 128x128 array, so four
matmul streams run CONCURRENTLY via tile_position=(32s, 0) — column slice q
of each PSUM tile runs in strip q. This quadruples matmul throughput and
lets each strip's LDWEIGHTS prefetch under the other strips' matmuls.

The 4-strip layout also makes every input DMA a dense 128-partition
transfer (the fast path: ~300 GB/s vs ~50 GB/s for a 34-partition load): B
is packed on the host so strip s's rows hold exactly the columns strip s
consumes (no duplication), A is replicated per strip, and the fp32 ACT bias
rides along bit-cast as bf16 column pairs. The two input loads go on the
two parallel HWDGE rings (sync + scalar); output stores alternate between
the rings, and the last two row-blocks store in PSW halves right after
each activation so the tail transfers start as early as possible.

walrus in this container rejects instructions carrying more than one
semaphore wait, which shapes several things: single fused input tensors
(first matmul = one wait), persistent PSUM tiles (pool re-allocation adds a
same-engine PE wait), total HWDGE DMA count <= 8 (lane reuse adds an
ordering wait), ACT->ACT pseudo-deps demoted to nosync, and a chain of
single-wait NOPs on the sync sequencer that "observes" every completion
before the framework's kernel-tail drain (the nops nosync-order after all
DMAs so none head-of-line blocks a pending store).
"""

import numpy as np
import ml_dtypes

import concourse.bass as bass
import concourse.mybir as mybir
import concourse.tile as tile
from concourse.bass_utils import run_bass_kernel_spmd
from concourse.tile import add_dep_helper

N, M, P = 4096, 4096, 32
NCORES = 8
NSHARD = N // NCORES  # 512 rows of x1 per core
IBLK = 128            # output row-block = PSUM partition dim
JBLK = 512            # matmul free dim = one fp32 PSUM bank
PSW = 2048            # PSUM tile width (4 banks) = one exp-activation
NSTRIP = 4            # concurrent PE row strips (K=32 each)
NI = NSHARD // IBLK   # 4 row-blocks
NH = M // PSW         # 2 PSUM tiles per row-block

BF16 = mybir.dt.bfloat16
NPBF16 = ml_dtypes.bfloat16

# load1 (sync ring) carries only what row-block i=0 consumes, so the
# saturated ACT chain starts as early as possible; the A slices for
# i=1..3 ride the otherwise-idle SWDGE channel (load3) — slow (~60 GB/s)
# but they land well before their first use.
#   l1: [A_hi(i0) 128 | A_lo(i0) 128 | Bh(h=0) 512 | Bl(h=0) 512 |
#        Yn(h=0) 512 | ones 128 | xn_bits 8 | pad 56]         -> 1984 cols
#   l2: [Bh(h=1) 512 | Bl(h=1) 512 | Yn(h=1) 512 | pad 64]    -> 1600 cols
#   l3: [A_hi(i123) 384 | A_lo(i123) 384]                     -> 768 cols
AHI_O = 0
ALO_O = 128
BH0_O = 256
BL0_O = 768
YN0_O = 1280
ONES_O = 1792
XN_O = 1920
L1_W = 1984
L2_W = 1600
AHI3_O = 0
ALO3_O = 384
L3_W = 768


def _build_nc():
    nc = bass.Bass()
    l1_t = nc.declare_dram_parameter("l1_t", [IBLK, L1_W], BF16, isOutput=False)
    l2_t = nc.declare_dram_parameter("l2_t", [IBLK, L2_W], BF16, isOutput=False)
    l3_t = nc.declare_dram_parameter("l3_t", [IBLK, L3_W], BF16, isOutput=False)
    out = nc.declare_dram_parameter("out", [NSHARD, M], mybir.dt.float32, isOutput=True)

    with tile.TileContext(nc) as tc:
        with (
            tc.tile_pool(name="inp", bufs=1) as inp_pool,
            tc.tile_pool(name="stage", bufs=1) as stage_pool,
            tc.tile_pool(name="ps", bufs=1, space="PSUM") as ps_pool,
        ):
            dma_insts = []
            l1_sb = inp_pool.tile([IBLK, L1_W], BF16, tag="l1")
            dma_insts.append(nc.sync.dma_start(out=l1_sb, in_=l1_t[:, :]))
            l2_sb = inp_pool.tile([IBLK, L2_W], BF16, tag="l2")
            dma_insts.append(nc.scalar.dma_start(out=l2_sb, in_=l2_t[:, :]))
            l3_sb = inp_pool.tile([IBLK, L3_W], BF16, tag="l3")
            dma_insts.append(nc.gpsimd.dma_start(out=l3_sb, in_=l3_t[:, :]))

            def rows(s, k=32):
                return slice(32 * s, 32 * s + k)

            def bh(h, s):
                sb, o = (l1_sb, BH0_O) if h == 0 else (l2_sb, 0)
                return sb[rows(s), o : o + JBLK]

            def bl(h, s):
                sb, o = (l1_sb, BL0_O) if h == 0 else (l2_sb, JBLK)
                return sb[rows(s), o : o + JBLK]

            def ynr(h, s):
                sb, o = (l1_sb, YN0_O) if h == 0 else (l2_sb, 2 * JBLK)
                return sb[rows(s, 3), o : o + JBLK]

            xn_bias = l1_sb[:, XN_O : XN_O + 2 * NI].bitcast(mybir.dt.float32)

            # Tiny ACT-engine read of l1 so the scalar engine observes the l1
            # DMA semaphore here (1 wait); the real activations then carry
            # only their PE wait (walrus rejects multi-wait ACTIVATE, and
            # Tile doesn't track that the PE wait transitively covers l1).
            scratch = inp_pool.tile([IBLK, 1], mybir.dt.float32, tag="scr")
            nc.scalar.copy(out=scratch, in_=l1_sb[:, 0:1])

            ps_tiles = [
                ps_pool.tile(
                    [IBLK, PSW], mybir.dt.float32, tag=f"ps{h}", name=f"ps{h}"
                )
                for h in range(NH)
            ]

            act_insts = []
            mm_insts = []
            for i in range(NI):
                out_sb = stage_pool.tile(
                    [IBLK, M], mybir.dt.float32, tag=f"out{i}", name=f"out{i}"
                )
                for h in range(NH):
                    ps = ps_tiles[h]
                    # 4 passes x 4 strips; strip s = column slice q=s of the
                    # PSUM tile. Inner loop cycles strips so consecutive
                    # matmuls run in different row groups (concurrent).
                    if i == 0:
                        ahi_sb, ahi_o = l1_sb, AHI_O
                        alo_sb, alo_o = l1_sb, ALO_O
                    else:
                        ahi_sb, ahi_o = l3_sb, AHI3_O + (i - 1) * IBLK
                        alo_sb, alo_o = l3_sb, ALO3_O + (i - 1) * IBLK
                    for p in range(4):
                        start = p == 0
                        stop = p == 3
                        for s in range(NSTRIP):
                            if p == 0:
                                lhsT = ahi_sb[rows(s), ahi_o : ahi_o + IBLK]
                                rhs = bh(h, s)
                            elif p == 1:
                                lhsT = ahi_sb[rows(s), ahi_o : ahi_o + IBLK]
                                rhs = bl(h, s)
                            elif p == 2:
                                lhsT = alo_sb[rows(s), alo_o : alo_o + IBLK]
                                rhs = bh(h, s)
                            else:
                                lhsT = l1_sb[rows(s, 3), ONES_O : ONES_O + IBLK]
                                rhs = ynr(h, s)
                            mm_insts.append(
                                nc.tensor.matmul(
                                    ps[:, s * JBLK : (s + 1) * JBLK],
                                    lhsT=lhsT,
                                    rhs=rhs,
                                    start=start,
                                    stop=stop,
                                    tile_position=(32 * s, 0),
                                )
                            )
                    act_insts.append(
                        nc.scalar.activation(
                            out=out_sb[:, h * PSW : (h + 1) * PSW],
                            in_=ps,
                            func=mybir.ActivationFunctionType.Exp,
                            bias=xn_bias[:, i : i + 1],
                            scale=1.0,
                        )
                    )
                    # Last two row-blocks: store each PSW half as soon as its
                    # activation lands (the early blocks' full-width stores
                    # already overlap compute; the tail ones wouldn't). Total
                    # DMA count stays at 8 = one per HWDGE lane.
                    if i >= NI - 2:
                        eng = nc.sync if (i + h) % 2 == 0 else nc.scalar
                        dma_insts.append(
                            eng.dma_start(
                                out=out[
                                    i * IBLK : (i + 1) * IBLK,
                                    h * PSW : (h + 1) * PSW,
                                ],
                                in_=out_sb[:, h * PSW : (h + 1) * PSW],
                            )
                        )
                if i < NI - 2:
                    eng = nc.sync if i % 2 == 0 else nc.scalar
                    dma_insts.append(
                        eng.dma_start(
                            out=out[i * IBLK : (i + 1) * IBLK, :], in_=out_sb
                        )
                    )

            # Demote ACT->ACT pseudo-deps (PSUM bank read-read serialization,
            # already ordered through the interleaved matmuls + same-engine
            # FIFO) to nosync: walrus rejects multi-wait ACTIVATE.
            import bass_rust as _br

            act_names = {a.ins.name for a in act_insts}
            for a in act_insts:
                deps = list(a.ins.sync_dependency_names())
                spurious = [d for d in deps if d in act_names]
                if spurious:
                    keep = [d for d in deps if d not in act_names]
                    a.ins.take_sync_dependencies()
                    a.ins.set_sync_dependencies(
                        _br.InstructionNameOrderedSet(keep)
                    )
                    a.ins.add_nosync_dependencies_from(
                        _br.InstructionNameOrderedSet(spurious)
                    )

            # Wait-funnel so the framework's kernel-tail drain needs no waits
            # of its own (walrus rejects its usual all-sems wait list). Each
            # nop also nosync-orders after every DMA so the scheduler cannot
            # slot a slow-waiting nop ahead of a still-pending store on the
            # same queue (head-of-line blocking).
            for t in [mm_insts[-1], act_insts[-1], *dma_insts]:
                nop = nc.sync.nop(nofuse=True, hint="tail_funnel")
                add_dep_helper(nop.ins, t.ins, True, "tail wait funnel")
                for dd in dma_insts:
                    if dd is not t:
                        add_dep_helper(nop.ins, dd.ins, False, "funnel order")
    return nc


def _bf16_splits(x, n):
    """Split fp32 array into n bf16 parts summing to ~x."""
    parts = []
    rem = x.astype(np.float32)
    for _ in range(n):
        p = rem.astype(NPBF16)
        parts.append(p)
        rem = rem - p.astype(np.float32)
    return parts


def run(x1, x2, log_rho, log_sigma, trace=False):
    """Returns (K, exec_time_ns). exec_time_ns is None unless trace=True."""
    x1 = np.asarray(x1, dtype=np.float32)
    x2 = np.asarray(x2, dtype=np.float32)
    rho = float(np.exp(np.float64(np.asarray(log_rho))))
    log_sig = 2.0 * float(np.asarray(log_sigma))  # log(sigma)

    xs = (x1 / np.float32(rho)).astype(np.float32)
    ys = (x2 / np.float32(rho)).astype(np.float32)
    xn = np.einsum("np,np->n", xs, xs, dtype=np.float64)
    yn = np.einsum("mp,mp->m", ys, ys, dtype=np.float64)

    a = xs.T.astype(np.float32)  # (32, N)
    b = ys.T.astype(np.float32)  # (32, M)
    a_hi, a_lo = _bf16_splits(a, 2)
    b_hi, b_lo = _bf16_splits(b, 2)
    y1, y2, y3 = _bf16_splits((-0.5 * yn).astype(np.float32), 3)
    # per-row ACT bias: -0.5*||x_i||^2 + log(sigma), exact fp32
    xbias = ((-0.5 * xn) + log_sig).astype(np.float32)

    def pack_b(src, h):
        # strip s rows hold the columns strip s consumes: B[:, h*PSW+s*JBLK..]
        o = np.zeros((IBLK, JBLK), NPBF16)
        for s in range(NSTRIP):
            o[32 * s : 32 * s + 32] = src[:, h * PSW + s * JBLK : h * PSW + (s + 1) * JBLK]
        return o

    def pack_yn(h):
        o = np.zeros((IBLK, JBLK), NPBF16)
        for s in range(NSTRIP):
            for r, yr in enumerate((y1, y2, y3)):
                o[32 * s + r] = yr[h * PSW + s * JBLK : h * PSW + (s + 1) * JBLK]
        return o

    ones = np.zeros((IBLK, IBLK), NPBF16)
    for s in range(NSTRIP):
        ones[32 * s : 32 * s + 3] = NPBF16(1.0)

    l2 = np.zeros((IBLK, L2_W), NPBF16)
    l2[:, 0:JBLK] = pack_b(b_hi, 1)
    l2[:, JBLK : 2 * JBLK] = pack_b(b_lo, 1)
    l2[:, 2 * JBLK : 3 * JBLK] = pack_yn(1)

    nc = _build_nc()
    in_maps = []
    for c in range(NCORES):
        s0 = slice(c * NSHARD, c * NSHARD + IBLK)
        s123 = slice(c * NSHARD + IBLK, (c + 1) * NSHARD)
        l1 = np.zeros((IBLK, L1_W), NPBF16)
        l3 = np.zeros((IBLK, L3_W), NPBF16)
        for s in range(NSTRIP):
            r = slice(32 * s, 32 * s + 32)
            l1[r, AHI_O : AHI_O + IBLK] = a_hi[:, s0]
            l1[r, ALO_O : ALO_O + IBLK] = a_lo[:, s0]
            l3[r, AHI3_O : AHI3_O + 3 * IBLK] = a_hi[:, s123]
            l3[r, ALO3_O : ALO3_O + 3 * IBLK] = a_lo[:, s123]
        l1[:, BH0_O : BH0_O + JBLK] = pack_b(b_hi, 0)
        l1[:, BL0_O : BL0_O + JBLK] = pack_b(b_lo, 0)
        l1[:, YN0_O : YN0_O + JBLK] = pack_yn(0)
        l1[:, ONES_O : ONES_O + IBLK] = ones
        # fp32 bias bits ride along as bf16 column pairs
        xb = np.zeros((IBLK, NI), np.float32)
        for i in range(NI):
            xb[:, i] = xbias[c * NSHARD + i * IBLK : c * NSHARD + (i + 1) * IBLK]
        l1[:, XN_O : XN_O + 2 * NI] = xb.view(np.uint16).view(NPBF16)
        in_maps.append(
            {
                "l1_t": np.ascontiguousarray(l1),
                "l2_t": l2,
                "l3_t": np.ascontiguousarray(l3),
            }
        )

    res = run_bass_kernel_spmd(
        nc, in_maps, core_ids=list(range(NCORES)), trace=trace
    )
    full = np.concatenate(
        [res.results[c]["out"] for c in range(NCORES)], axis=0
    )
    return full, res.exec_time_ns


def kernel(x1, x2, log_rho, log_sigma):
    out, _ = run(x1, x2, log_rho, log_sigma, trace=False)
    return out
